# revision 27
# baseline (speedup 1.0000x reference)
"""Trainium2 Bass kernel for nn_Memory (scatter_memory).

Computation (see reference): l2-normalize query over C, score = qr @ keys.T,
double softmax, top-2 memory slots, triplet/compactness losses, attention
read (sm @ keys), segment-sum memory update + l2norm.

Strategy: data-parallel over flattened query axis N=32768 (4096 rows/core,
8 cores), keys replicated. Cross-core reductions via 3 AllReduces:
  AR1: colsumE [512]            (add)  -> score_query denominator
  AR2: segsum' [512,512]+losses (add)  -> memory update + loss scalars
  AR3: colmaxE [512]            (max)  -> segment weight normalization
Key algebra:
  softmax over N: sq = exp(score)/colsumE (no max-subtract; score in [-18,19])
  w[n] = exp(score[n,i1] - colmax[i1]) => segsum rows rescaled by 1/colmaxE[m]
  segsum lhsT is onehot1*keys2[m] (byproduct of a fused gather op); the
  keys2[m] row-scale is divided out at the end.
  d_pos^2 = qrn2 + keys2[i1] - 2*score[n,i1] (+O(1e-4) eps terms dropped)
  score[n,i1] = ln(max E) from the top-8 instruction.
"""

import sys

if "/opt/trn_rl_repo" not in sys.path:
    sys.path.insert(0, "/opt/trn_rl_repo")

import numpy as np

import concourse.bass as bass
import concourse.bass_isa as bass_isa
import concourse.tile as tile
from concourse import bacc, mybir
from concourse.bass_utils import run_bass_kernel_spmd
from concourse.masks import make_identity

B, C, T, D = 16, 64, 32, 512
M = 512
NCORES = 8
N = B * C * T            # 32768
NSH = N // NCORES        # 4096 rows per core
TN = NSH // 128          # 32 tiles of [128, D] per core
BPC = B // NCORES        # 2 batch entries per core
TPB = TN // BPC          # 16 tiles per batch entry

f32 = mybir.dt.float32
f32r = mybir.dt.float32r
bf16 = mybir.dt.bfloat16
u16 = mybir.dt.uint16
OP = mybir.AluOpType
ACT = mybir.ActivationFunctionType
AX = mybir.AxisListType

RG = [list(range(NCORES))]


def _emit(tc, q_in, k_in, sm_out, sq_out, uq_out, um_out, loss_out):
    nc = tc.nc

    q_dram = q_in.ap().rearrange("(t p) d -> p t d", p=128)    # [128, TN, D]
    k_dram = k_in.ap().rearrange("(j p) d -> p j d", p=128)    # [128, 4, D]
    sm_dram = sm_out.ap().rearrange("(t p) m -> p t m", p=128)
    sq_dram = sq_out.ap().rearrange("(t p) m -> p t m", p=128)
    uq_dram = uq_out.ap().rearrange("(t p) d -> p t d", p=128)
    um_dram = um_out.ap().rearrange("(j p) d -> p j d", p=128)

    from contextlib import ExitStack

    ctx = ExitStack()
    with ctx:
        const = ctx.enter_context(tc.tile_pool(name="const", bufs=1))
        big = ctx.enter_context(tc.tile_pool(name="big", bufs=1))
        stats = ctx.enter_context(tc.tile_pool(name="stats", bufs=1))
        dram = ctx.enter_context(tc.tile_pool(name="dram", bufs=1, space="DRAM"))

        # ---------- constants ----------
        identity = const.tile([128, 128], f32)
        make_identity(nc, identity[:])
        identity_bf = const.tile([128, 128], bf16)
        make_identity(nc, identity_bf[:])
        iota_u = const.tile([128, M], u16)
        nc.gpsimd.iota(iota_u[:], pattern=[[1, M]], base=0, channel_multiplier=0)
        ones_f = const.tile([128, 1], f32)
        nc.gpsimd.memset(ones_f[:], 1.0)
        ones_col = const.tile([128, 1], f32r)
        nc.scalar.copy(ones_col[:], ones_f[:])
        ones_bf = const.tile([128, 1], bf16)
        nc.scalar.copy(ones_bf[:], ones_f[:])

        # ---------- big resident buffers ----------
        q_res = big.tile([128, TN, D], f32)      # raw query rows
        E_res = big.tile([128, TN, M], bf16)     # exp(score) rows
        k_nat = big.tile([128, 4, D], f32r)      # keys  [m-chunk layout]
        k_nbf = big.tile([128, 4, D], bf16)      # keys in bf16 (uq matmul rhs)
        keysT = big.tile([128, 4, M], f32r)      # keys.T [d-chunk layout]
        inv_rep = big.tile([128, BPC, D], f32)   # 1/norm_C replicated x4

        # ---------- stats ----------
        vmax_all = stats.tile([128, TN, 8], f32)
        vidx_all = stats.tile([128, TN, 8], u16)
        rowinv_all = stats.tile([128, TN], f32)
        qrn2_all = stats.tile([128, TN], f32)
        A1_all = stats.tile([128, TN], f32)      # keys2[i1]
        A2_all = stats.tile([128, TN], f32)      # keys2[i2]
        colmax_acc = stats.tile([128, M], bf16)
        nc.gpsimd.memset(colmax_acc[:], 0.0)
        colmax_f = stats.tile([128, M], f32)
        keys2_col = stats.tile([128, 4], f32)    # keys2 in column layout (f32)
        keys2_bc = stats.tile([128, M], bf16)    # keys2 broadcast (gathers)
        colinv_bc = stats.tile([128, M], f32)    # 1/colsumE broadcast
        colmax_col = stats.tile([128, 4], f32)
        lossb = stats.tile([128, 2], f32)

        # ---------- DRAM bounce buffers for collectives ----------
        cc2_in = dram.tile([263168], f32)
        cc2_out = dram.tile([263168], f32, addr_space="Shared")
        cc2_in_v = cc2_in[0:262656].rearrange("(p x) -> p x", x=2052)
        cc2_out_v = cc2_out[0:262656].rearrange("(p x) -> p x", x=2052)
        cc3_in = dram.tile([1, M], f32)
        cc3_out = dram.tile([1, M], f32, addr_space="Shared")

        # ---------- working pools ----------
        wk_sq = ctx.enter_context(tc.tile_pool(name="wk_sq", bufs=2))
        wk_qn = ctx.enter_context(tc.tile_pool(name="wk_qn", bufs=2))
        wk_tr = ctx.enter_context(tc.tile_pool(name="wk_tr", bufs=3))
        wk_oh = ctx.enter_context(tc.tile_pool(name="wk_oh", bufs=3))
        wk_wqr = ctx.enter_context(tc.tile_pool(name="wk_wqr", bufs=2))
        st_sm = ctx.enter_context(tc.tile_pool(name="st_sm", bufs=2))
        st_uq = ctx.enter_context(tc.tile_pool(name="st_uq", bufs=2))
        small = ctx.enter_context(tc.tile_pool(name="small", bufs=1))

        selector_f = wk_sq.tile([128, TPB * BPC], f32, tag="wsq", name="sel_f")
        nc.gpsimd.memset(selector_f[:], 0.0)
        for k in range(4):
            nc.gpsimd.affine_select(
                out=selector_f[:], in_=selector_f[:],
                pattern=[[-1, 32]], compare_op=OP.not_equal, fill=1.0,
                base=-32 * k, channel_multiplier=1)
        selector = const.tile([128, TPB * BPC], f32r)
        nc.scalar.copy(selector[:], selector_f[:])

        # ---------- PSUM pools ----------
        pp_cs = ctx.enter_context(tc.tile_pool(name="pp_cs", bufs=1, space="PSUM"))

        # ================= PROLOGUE =================
        with tc.tile_pool(name="pp_pro", bufs=2, space="PSUM") as pp_pro, \
             tc.tile_pool(name="pp_norm", bufs=1, space="PSUM") as pp_norm:
            # keys (compute-rounded to f32r) + bf16 copy + keysT + keys2
            for jm in range(4):
                kscr = wk_qn.tile([128, D], f32, tag="wqn", name="kscr")
                nc.sync.dma_start(kscr[:], k_dram[:, jm, :])
                nc.scalar.copy(k_nat[:, jm, :], kscr[:])
                nc.vector.tensor_copy(k_nbf[:, jm, :], kscr[:])
            for jm in range(4):
                kt_ps = pp_pro.tile([128, M], f32, tag="kt")
                for jd in range(4):
                    nc.tensor.transpose(kt_ps[:, jd * 128:(jd + 1) * 128],
                                        k_nat[:, jm, jd * 128:(jd + 1) * 128].bitcast(f32),
                                        identity[:])
                for jd in range(4):
                    if jd < 2:
                        nc.scalar.copy(keysT[:, jd, jm * 128:(jm + 1) * 128],
                                       kt_ps[:, jd * 128:(jd + 1) * 128])
                    else:
                        nc.vector.tensor_copy(keysT[:, jd, jm * 128:(jm + 1) * 128],
                                              kt_ps[:, jd * 128:(jd + 1) * 128])

            # keys2 column layout [128, 4] (for updated_memory rescale)
            for jm in range(4):
                k2scr = wk_sq.tile([128, D], f32, tag="wsq", name="k2scr")
                nc.scalar.activation(k2scr[:], k_nat[:, jm, :].bitcast(f32),
                                     ACT.Square, accum_out=keys2_col[:, jm:jm + 1])
            # keys2 row layout -> bf16 broadcast (for gathers)
            k2row_ps = pp_cs.tile([1, M], f32, tag="cs")
            for jd in range(4):
                sqkT = wk_qn.tile([128, M], f32r, tag="wqn", name="sqkT")
                nc.scalar.activation(sqkT[:], keysT[:, jd, :].bitcast(f32), ACT.Square)
                nc.tensor.matmul(k2row_ps[:], ones_col[:], sqkT[:],
                                 start=(jd == 0), stop=(jd == 3))
            k2row = small.tile([1, M], bf16)
            nc.scalar.copy(k2row[:], k2row_ps[:])
            nc.gpsimd.partition_broadcast(keys2_bc[:], k2row[:])

            # query load
            for g in range(TN // 4):
                nc.sync.dma_start(q_res[:, 4 * g:4 * g + 4, :],
                                  q_dram[:, 4 * g:4 * g + 4, :])

            # per-b channel norms:  norm2[b, t, d] = sum_c query^2
            norm_ps = [pp_norm.tile([32, D], f32, tag=f"n{b}", name=f"norm{b}")
                       for b in range(BPC)]
            for b in range(BPC):
                for lt in range(TPB):
                    i = b * TPB + lt
                    sq_t = wk_sq.tile([128, D], f32r, tag="wsqr", name="sq_t")
                    if i % 2 == 0:
                        nc.scalar.activation(sq_t[:], q_res[:, i, :], ACT.Square)
                    else:
                        nc.vector.tensor_tensor(sq_t[:], q_res[:, i, :],
                                                q_res[:, i, :], OP.mult)
                    nc.tensor.matmul(norm_ps[b][:], selector[:], sq_t[:],
                                     start=(lt == 0), stop=(lt == TPB - 1))
            for b in range(BPC):
                nrm = small.tile([32, D], f32, tag="k2row", name="nrm")
                nc.scalar.sqrt(nrm[:], norm_ps[b][:])
                nc.vector.tensor_scalar_max(nrm[:], nrm[:], 1e-12)
                nc.vector.reciprocal(nrm[:], nrm[:])
                for k in range(4):
                    if k < 2:
                        nc.scalar.copy(inv_rep[32 * k:32 * k + 32, b, :], nrm[:])
                    else:
                        nc.vector.tensor_copy(inv_rep[32 * k:32 * k + 32, b, :], nrm[:])

        pp_a = ctx.enter_context(tc.tile_pool(name="pp_a", bufs=1, space="PSUM"))
        pp_b = ctx.enter_context(tc.tile_pool(name="pp_b", bufs=2, space="PSUM"))
        pp_seg = ctx.enter_context(tc.tile_pool(name="pp_seg", bufs=1, space="PSUM"))

        # ================= PHASE 1 (4-tile groups) =================
        colsum_ps = pp_cs.tile([1, M], f32, tag="cs")
        for g in range(TN // 4):
            for t in range(4):
                i = 4 * g + t
                b = i // TPB
                # qn overwrites the raw query rows in place
                nc.vector.scalar_tensor_tensor(q_res[:, i, :], q_res[:, i, :],
                                               1.0, inv_rep[:, b, :],
                                               OP.mult, OP.mult)
                qnT_ps = pp_a.tile([128, D], f32, tag="pa", name="qnT_ps")
                for jd in range(4):
                    nc.tensor.transpose(qnT_ps[:, jd * 128:(jd + 1) * 128],
                                        q_res[:, i, jd * 128:(jd + 1) * 128],
                                        identity[:])
                qnT_sb = wk_tr.tile([128, 4, 128], f32r, tag="wtr", name="qnT_sb")
                for jd in range(4):
                    if jd < 2:
                        nc.scalar.copy(qnT_sb[:, jd, :],
                                       qnT_ps[:, jd * 128:(jd + 1) * 128])
                    else:
                        nc.vector.tensor_copy(qnT_sb[:, jd, :],
                                              qnT_ps[:, jd * 128:(jd + 1) * 128])
                score_ps = pp_b.tile([128, M], f32, tag="pb", name="score_ps")
                for jd in range(4):
                    nc.tensor.matmul(score_ps[:], qnT_sb[:, jd, :],
                                     keysT[:, jd, :],
                                     start=(jd == 0), stop=(jd == 3))
                nc.scalar.activation(E_res[:, i, :], score_ps[:], ACT.Exp,
                                     accum_out=rowinv_all[:, i:i + 1])
                nc.tensor.matmul(colsum_ps[:], ones_bf[:], E_res[:, i, :],
                                 start=(i == 0), stop=(i == TN - 1),
                                 skip_group_check=True)
                nc.vector.max(vmax_all[:, i, :], E_res[:, i, :])
                nc.vector.max_index(vidx_all[:, i, :], vmax_all[:, i, :],
                                    E_res[:, i, :])
                nc.vector.tensor_tensor(colmax_acc[:], colmax_acc[:],
                                        E_res[:, i, :], OP.max)
                sq2 = wk_sq.tile([128, D], f32, tag="wsq", name="sq2")
                nc.scalar.activation(sq2[:], q_res[:, i, :], ACT.Square,
                                     accum_out=qrn2_all[:, i:i + 1])
            # batched reciprocal + sm for the group
            nc.vector.reciprocal(rowinv_all[:, 4 * g:4 * g + 4],
                                 rowinv_all[:, 4 * g:4 * g + 4])
            sm_stage = st_sm.tile([128, 4, M], f32, tag="ssm", name="sm_stage")
            for t in range(4):
                i = 4 * g + t
                nc.scalar.activation(sm_stage[:, t, :], E_res[:, i, :], ACT.Copy,
                                     scale=rowinv_all[:, i:i + 1])
            nc.sync.dma_start(sm_dram[:, 4 * g:4 * g + 4, :], sm_stage[:])
            # attention read (uq = sm @ keys), interleaved for PE density
            uq_stage = st_uq.tile([128, 4, D], f32, tag="suq", name="uq_stage")
            for t in range(4):
                i = 4 * g + t
                ET_ps = pp_a.tile([128, M], bf16, tag="pa", name="ET_ps")
                for jm in range(4):
                    nc.tensor.transpose(ET_ps[:, jm * 128:(jm + 1) * 128],
                                        E_res[:, i, jm * 128:(jm + 1) * 128],
                                        identity_bf[:])
                ET_sb = wk_tr.tile([128, 4, 128], bf16, tag="wtr2", name="ET_sb")
                for jm in range(4):
                    nc.scalar.copy(ET_sb[:, jm, :],
                                   ET_ps[:, jm * 128:(jm + 1) * 128])
                uq_ps = pp_b.tile([128, D], f32, tag="pb", name="uq_ps")
                for jm in range(4):
                    nc.tensor.matmul(uq_ps[:], ET_sb[:, jm, :], k_nbf[:, jm, :],
                                     start=(jm == 0), stop=(jm == 3))
                nc.scalar.activation(uq_stage[:, t, :], uq_ps[:], ACT.Copy,
                                     scale=rowinv_all[:, i:i + 1])
            nc.sync.dma_start(uq_dram[:, 4 * g:4 * g + 4, :], uq_stage[:])

        # colsum -> cc2 (column layout: m = 128j + p at [p, 2052+j])
        csrow = small.tile([1, M], f32)
        nc.scalar.copy(csrow[:], colsum_ps[:])
        nc.sync.dma_start(cc2_in[262656:263168].rearrange("(x m) -> x m", x=1),
                          csrow[:])
        # column max of E (partition reduce; collective after AR2)
        nc.gpsimd.partition_all_reduce(colmax_f[:], colmax_acc[:], 128,
                                       bass_isa.ReduceOp.max)
        cmrow = small.tile([1, M], f32)
        nc.vector.tensor_copy(cmrow[:], colmax_f[0:1, :])
        nc.sync.dma_start(cc3_in[:], cmrow[:])

        # ================= PHASE 2a-seg: segsum first (early AR2) ==========
        seg_ps = [pp_seg.tile([128, D], f32, tag=f"sg{j}", name=f"seg{j}")
                  for j in range(4)]
        for i in range(TN):
            oh1k = wk_oh.tile([128, M], bf16, tag="woh", name="oh1k")
            nc.vector.scalar_tensor_tensor(oh1k[:], iota_u[:],
                                           vidx_all[:, i, 0:1], keys2_bc[:],
                                           OP.is_equal, OP.mult,
                                           accum_out=A1_all[:, i:i + 1])
            oh2k = wk_oh.tile([128, M], bf16, tag="woh", name="oh2k")
            nc.vector.scalar_tensor_tensor(oh2k[:], iota_u[:],
                                           vidx_all[:, i, 1:2], keys2_bc[:],
                                           OP.is_equal, OP.mult,
                                           accum_out=A2_all[:, i:i + 1])
            # wqr = vmax0 * qn   (qn lives in q_res now)
            wqr = wk_wqr.tile([128, D], bf16, tag="wwqr", name="wqr")
            nc.vector.tensor_scalar_mul(wqr[:], q_res[:, i, :],
                                        vmax_all[:, i, 0:1])
            for jm in range(4):
                nc.tensor.matmul(seg_ps[jm][:], oh1k[:, jm * 128:(jm + 1) * 128],
                                 wqr[:], start=(i == 0), stop=(i == TN - 1),
                                 skip_group_check=True)

        # ---- batched losses (A1/A2 complete) ----
        lnv = small.tile([128, TN, 2], f32)
        nc.scalar.activation(lnv[:], vmax_all[:, :, 0:2], ACT.Ln)
        dp2 = small.tile([128, TN], f32)
        dn2 = small.tile([128, TN], f32)
        nc.vector.scalar_tensor_tensor(dp2[:], lnv[:, :, 0], -2.0, A1_all[:],
                                       OP.mult, OP.add)
        nc.vector.tensor_tensor(dp2[:], dp2[:], qrn2_all[:], OP.add)
        nc.vector.scalar_tensor_tensor(dn2[:], lnv[:, :, 1], -2.0, A2_all[:],
                                       OP.mult, OP.add)
        nc.vector.tensor_tensor(dn2[:], dn2[:], qrn2_all[:], OP.add)
        cscr = small.tile([128, TN], f32)
        nc.vector.tensor_scalar(cscr[:], dp2[:], 1.0, 0.0, OP.mult, OP.add,
                                accum_out=lossb[:, 1:2])
        nc.scalar.sqrt(dp2[:], dp2[:])
        nc.scalar.sqrt(dn2[:], dn2[:])
        nc.vector.scalar_tensor_tensor(cscr[:], dn2[:], -1.0, dp2[:],
                                       OP.mult, OP.add)
        sscr = small.tile([128, TN], f32)
        nc.scalar.activation(sscr[:], cscr[:], ACT.Relu, bias=1.0,
                             accum_out=lossb[:, 0:1])

        # ---- AR2: segsum + losses (overlaps the uq sweep below) ----
        seg_sb = q_res[:, 0:4, :]
        for jm in range(4):
            if jm < 2:
                nc.scalar.copy(seg_sb[:, jm, :], seg_ps[jm][:])
            else:
                nc.vector.tensor_copy(seg_sb[:, jm, :], seg_ps[jm][:])
        nc.sync.dma_start(cc2_in_v[:, 0:2048], seg_sb[:])
        nc.sync.dma_start(cc2_in_v[:, 2048:2050], lossb[:])
        nc.gpsimd.collective_compute("AllReduce", OP.add, replica_groups=RG,
                                     ins=[cc2_in[:].opt()], outs=[cc2_out[:].opt()])
        nc.gpsimd.collective_compute("AllReduce", OP.max, replica_groups=RG,
                                     ins=[cc3_in[:].opt()], outs=[cc3_out[:].opt()])

        # ================= PHASE 2b: score_query out =================
        nc.sync.dma_start(csrow[:],
                          cc2_out[262656:263168].rearrange("(x m) -> x m", x=1))
        nc.vector.reciprocal(csrow[:], csrow[:])
        nc.gpsimd.partition_broadcast(colinv_bc[:], csrow[:])
        sq_stage = None
        for i in range(TN):
            if i % 4 == 0:
                sq_stage = st_sm.tile([128, 4, M], f32, tag="ssm", name="sq_stage")
            nc.vector.tensor_tensor(sq_stage[:, i % 4, :], E_res[:, i, :],
                                    colinv_bc[:], OP.mult)
            if i % 4 == 3:
                nc.sync.dma_start(sq_dram[:, i - 3:i + 1, :], sq_stage[:])

        # ================= TAIL: updated_memory + losses =================
        nc.sync.dma_start(colmax_col[:],
                          cc3_out[:].rearrange("x (j p) -> p (x j)", p=128))
        # rescale = 1/(keys2[m] * colmaxE[m])
        nc.vector.tensor_tensor(colmax_col[:], colmax_col[:], keys2_col[:], OP.mult)
        nc.vector.reciprocal(colmax_col[:], colmax_col[:])
        seg_glob = q_res[:, 4:8, :]
        nc.sync.dma_start(seg_glob[:], cc2_out_v[:, 0:2048])
        umn = small.tile([128, 4], f32)
        for jm in range(4):
            nc.vector.scalar_tensor_tensor(seg_glob[:, jm, :], seg_glob[:, jm, :],
                                           colmax_col[:, jm:jm + 1],
                                           k_nat[:, jm, :].bitcast(f32),
                                           OP.mult, OP.add)
            umscr = wk_sq.tile([128, D], f32, tag="wsq", name="umscr")
            nc.scalar.activation(umscr[:], seg_glob[:, jm, :], ACT.Square,
                                 accum_out=umn[:, jm:jm + 1])
        nc.scalar.sqrt(umn[:], umn[:])
        nc.vector.tensor_scalar_max(umn[:], umn[:], 1e-12)
        nc.vector.reciprocal(umn[:], umn[:])
        for jm in range(4):
            nc.vector.tensor_scalar_mul(seg_glob[:, jm, :], seg_glob[:, jm, :],
                                        umn[:, jm:jm + 1])
        nc.sync.dma_start(um_dram[:], seg_glob[:])

        # global losses
        nc.sync.dma_start(lossb[:], cc2_out_v[:, 2048:2050])
        nc.gpsimd.partition_all_reduce(lossb[:], lossb[:], 128,
                                       bass_isa.ReduceOp.add)
        lrow = small.tile([1, 2], f32)
        nc.vector.tensor_copy(lrow[:], lossb[0:1, :])
        nc.sync.dma_start(loss_out.ap()[:], lrow[:])


_CACHE = {}


def _build():
    if "nc" in _CACHE:
        return _CACHE["nc"]
    nc = bacc.Bacc("TRN2", target_bir_lowering=False, debug=False,
                   num_devices=NCORES)
    q_in = nc.dram_tensor("q", [NSH, D], f32, kind="ExternalInput")
    k_in = nc.dram_tensor("k", [M, D], f32, kind="ExternalInput")
    sm_out = nc.dram_tensor("sm", [NSH, M], f32, kind="ExternalOutput")
    sq_out = nc.dram_tensor("sq", [NSH, M], f32, kind="ExternalOutput")
    uq_out = nc.dram_tensor("uq", [NSH, D], f32, kind="ExternalOutput")
    um_out = nc.dram_tensor("um", [M, D], f32, kind="ExternalOutput")
    loss_out = nc.dram_tensor("loss", [1, 2], f32, kind="ExternalOutput")
    with tile.TileContext(nc) as tc:
        _emit(tc, q_in, k_in, sm_out, sq_out, uq_out, um_out, loss_out)
    nc.compile()
    _CACHE["nc"] = nc
    return nc


def run(query, keys, trace=False, **trace_kw):
    nc = _build()
    qr = np.ascontiguousarray(np.asarray(query, np.float32).reshape(N, D))
    kk = np.ascontiguousarray(np.asarray(keys, np.float32))
    in_maps = [{"q": qr[c * NSH:(c + 1) * NSH], "k": kk} for c in range(NCORES)]
    res = run_bass_kernel_spmd(nc, in_maps, core_ids=list(range(NCORES)),
                               trace=trace, **trace_kw)
    return res


def kernel(query, keys):
    res = run(query, keys)
    outs = res.results
    uq = np.concatenate([outs[c]["uq"] for c in range(NCORES)], axis=0)
    sm = np.concatenate([outs[c]["sm"] for c in range(NCORES)], axis=0)
    sq = np.concatenate([outs[c]["sq"] for c in range(NCORES)], axis=0)
    um = outs[0]["um"]
    loss = outs[0]["loss"].reshape(2)
    updated_query = uq.reshape(B, C, T, D)
    separateness = np.float32(loss[0] / N)
    compactness = np.float32(loss[1] / (N * D))
    return (updated_query, um, sq, sm, separateness, compactness)


# revision 28
# speedup vs baseline: 1.2617x; 1.2617x over previous
"""Trainium2 Bass kernel for nn_Memory (scatter_memory).

Computation (see reference): l2-normalize query over C, score = qr @ keys.T,
double softmax, top-2 memory slots, triplet/compactness losses, attention
read (sm @ keys), segment-sum memory update + l2norm.

Strategy: data-parallel over flattened query axis N=32768 (4096 rows/core,
8 cores), keys replicated. Cross-core reductions via 3 AllReduces:
  AR1: colsumE [512]            (add)  -> score_query denominator
  AR2: segsum' [512,512]+losses (add)  -> memory update + loss scalars
  AR3: colmaxE [512]            (max)  -> segment weight normalization
Key algebra:
  softmax over N: sq = exp(score)/colsumE (no max-subtract; score in [-18,19])
  w[n] = exp(score[n,i1] - colmax[i1]) => segsum rows rescaled by 1/colmaxE[m]
  segsum lhsT is onehot1*keys2[m] (byproduct of a fused gather op); the
  keys2[m] row-scale is divided out at the end.
  d_pos^2 = qrn2 + keys2[i1] - 2*score[n,i1] (+O(1e-4) eps terms dropped)
  score[n,i1] = ln(max E) from the top-8 instruction.
"""

import sys

if "/opt/trn_rl_repo" not in sys.path:
    sys.path.insert(0, "/opt/trn_rl_repo")

import numpy as np

import concourse.bass as bass
import concourse.bass_isa as bass_isa
import concourse.tile as tile
from concourse import bacc, mybir
from concourse.bass_utils import run_bass_kernel_spmd
from concourse.masks import make_identity

B, C, T, D = 16, 64, 32, 512
M = 512
NCORES = 8
N = B * C * T            # 32768
NSH = N // NCORES        # 4096 rows per core
TN = NSH // 128          # 32 tiles of [128, D] per core
BPC = B // NCORES        # 2 batch entries per core
TPB = TN // BPC          # 16 tiles per batch entry

f32 = mybir.dt.float32
f32r = mybir.dt.float32r
bf16 = mybir.dt.bfloat16
u16 = mybir.dt.uint16
OP = mybir.AluOpType
ACT = mybir.ActivationFunctionType
AX = mybir.AxisListType

RG = [list(range(NCORES))]


def _emit(tc, q_in, k_in, sm_out, sq_out, uq_out, um_out, loss_out):
    nc = tc.nc

    q_dram = q_in.ap().rearrange("(t p) d -> p t d", p=128)    # [128, TN, D]
    k_dram = k_in.ap().rearrange("(j p) d -> p j d", p=128)    # [128, 4, D]
    sm_dram = sm_out.ap().rearrange("(t p) m -> p t m", p=128)
    sq_dram = sq_out.ap().rearrange("(t p) m -> p t m", p=128)
    uq_dram = uq_out.ap().rearrange("(t p) d -> p t d", p=128)
    um_dram = um_out.ap().rearrange("(j p) d -> p j d", p=128)

    from contextlib import ExitStack

    ctx = ExitStack()
    with ctx:
        const = ctx.enter_context(tc.tile_pool(name="const", bufs=1))
        big = ctx.enter_context(tc.tile_pool(name="big", bufs=1))
        stats = ctx.enter_context(tc.tile_pool(name="stats", bufs=1))
        dram = ctx.enter_context(tc.tile_pool(name="dram", bufs=1, space="DRAM"))

        # ---------- constants ----------
        identity = const.tile([128, 128], f32)
        make_identity(nc, identity[:])
        identity_bf = const.tile([128, 128], bf16)
        make_identity(nc, identity_bf[:])
        iota_u = const.tile([128, M], u16)
        nc.gpsimd.iota(iota_u[:], pattern=[[1, M]], base=0, channel_multiplier=0)
        ones_f = const.tile([128, 1], f32)
        nc.gpsimd.memset(ones_f[:], 1.0)
        ones_col = const.tile([128, 1], f32r)
        nc.scalar.copy(ones_col[:], ones_f[:])
        ones_bf = const.tile([128, 1], bf16)
        nc.scalar.copy(ones_bf[:], ones_f[:])

        # ---------- big resident buffers ----------
        q_res = big.tile([128, TN, D], f32)      # raw query rows
        E_res = big.tile([128, TN, M], bf16)     # exp(score) rows
        k_nat = big.tile([128, 4, D], f32r)      # keys  [m-chunk layout]
        k_nbf = big.tile([128, 4, D], bf16)      # keys in bf16 (uq matmul rhs)
        keysT = big.tile([128, 4, M], f32r)      # keys.T [d-chunk layout]
        inv_rep = big.tile([128, BPC, D], f32)   # 1/norm_C replicated x4

        # ---------- stats ----------
        vmax_all = stats.tile([128, TN, 8], f32)
        vidx_all = stats.tile([128, TN, 8], u16)
        rowinv_all = stats.tile([128, TN], f32)
        qrn2_all = stats.tile([128, TN], f32)
        A1_all = stats.tile([128, TN], f32)      # keys2[i1]
        A2_all = stats.tile([128, TN], f32)      # keys2[i2]
        colmax_acc = stats.tile([128, M], bf16)
        nc.gpsimd.memset(colmax_acc[:], 0.0)
        colmax_f = stats.tile([128, M], f32)
        keys2_col = stats.tile([128, 4], f32)    # keys2 in column layout (f32)
        keys2_bc = stats.tile([128, M], bf16)    # keys2 broadcast (gathers)
        colinv_bc = stats.tile([128, M], f32)    # 1/colsumE broadcast
        colmax_col = stats.tile([128, 4], f32)
        lossb = stats.tile([128, 2], f32)

        # ---------- DRAM bounce buffers for collectives ----------
        cc2_in = dram.tile([263168], f32)
        cc2_out = dram.tile([263168], f32, addr_space="Shared")
        cc2_in_v = cc2_in[0:262656].rearrange("(p x) -> p x", x=2052)
        cc2_out_v = cc2_out[0:262656].rearrange("(p x) -> p x", x=2052)
        cc3_in = dram.tile([1, M], f32)
        cc3_out = dram.tile([1, M], f32, addr_space="Shared")

        # ---------- working pools ----------
        wk_sq = ctx.enter_context(tc.tile_pool(name="wk_sq", bufs=2))
        wk_qn = ctx.enter_context(tc.tile_pool(name="wk_qn", bufs=2))
        wk_tr = ctx.enter_context(tc.tile_pool(name="wk_tr", bufs=3))
        wk_oh = ctx.enter_context(tc.tile_pool(name="wk_oh", bufs=3))
        wk_wqr = ctx.enter_context(tc.tile_pool(name="wk_wqr", bufs=2))
        st_sm = ctx.enter_context(tc.tile_pool(name="st_sm", bufs=2))
        st_uq = ctx.enter_context(tc.tile_pool(name="st_uq", bufs=2))
        small = ctx.enter_context(tc.tile_pool(name="small", bufs=1))

        selector_f = wk_sq.tile([128, TPB * BPC], f32, tag="wsq", name="sel_f")
        nc.gpsimd.memset(selector_f[:], 0.0)
        for k in range(4):
            nc.gpsimd.affine_select(
                out=selector_f[:], in_=selector_f[:],
                pattern=[[-1, 32]], compare_op=OP.not_equal, fill=1.0,
                base=-32 * k, channel_multiplier=1)
        selector = const.tile([128, TPB * BPC], f32r)
        nc.scalar.copy(selector[:], selector_f[:])

        # ---------- PSUM pools ----------
        pp_cs = ctx.enter_context(tc.tile_pool(name="pp_cs", bufs=1, space="PSUM"))

        # ================= PROLOGUE =================
        with tc.tile_pool(name="pp_pro", bufs=2, space="PSUM") as pp_pro, \
             tc.tile_pool(name="pp_norm", bufs=1, space="PSUM") as pp_norm:
            # keys (compute-rounded to f32r) + bf16 copy + keysT + keys2
            for jm in range(4):
                kscr = wk_qn.tile([128, D], f32, tag="wqn", name="kscr")
                nc.sync.dma_start(kscr[:], k_dram[:, jm, :])
                nc.scalar.copy(k_nat[:, jm, :], kscr[:])
                nc.vector.tensor_copy(k_nbf[:, jm, :], kscr[:])
            for jm in range(4):
                kt_ps = pp_pro.tile([128, M], f32, tag="kt")
                for jd in range(4):
                    nc.tensor.transpose(kt_ps[:, jd * 128:(jd + 1) * 128],
                                        k_nat[:, jm, jd * 128:(jd + 1) * 128].bitcast(f32),
                                        identity[:])
                for jd in range(4):
                    if jd < 2:
                        nc.scalar.copy(keysT[:, jd, jm * 128:(jm + 1) * 128],
                                       kt_ps[:, jd * 128:(jd + 1) * 128])
                    else:
                        nc.vector.tensor_copy(keysT[:, jd, jm * 128:(jm + 1) * 128],
                                              kt_ps[:, jd * 128:(jd + 1) * 128])

            # keys2 column layout [128, 4] (for updated_memory rescale)
            for jm in range(4):
                k2scr = wk_sq.tile([128, D], f32, tag="wsq", name="k2scr")
                nc.scalar.activation(k2scr[:], k_nat[:, jm, :].bitcast(f32),
                                     ACT.Square, accum_out=keys2_col[:, jm:jm + 1])
            # keys2 row layout -> bf16 broadcast (for gathers)
            k2row_ps = pp_cs.tile([1, M], f32, tag="cs")
            for jd in range(4):
                sqkT = wk_qn.tile([128, M], f32r, tag="wqn", name="sqkT")
                nc.scalar.activation(sqkT[:], keysT[:, jd, :].bitcast(f32), ACT.Square)
                nc.tensor.matmul(k2row_ps[:], ones_col[:], sqkT[:],
                                 start=(jd == 0), stop=(jd == 3))
            k2row = small.tile([1, M], bf16)
            nc.scalar.copy(k2row[:], k2row_ps[:])
            nc.gpsimd.partition_broadcast(keys2_bc[:], k2row[:])

            # query load
            for g in range(TN // 4):
                nc.sync.dma_start(q_res[:, 4 * g:4 * g + 4, :],
                                  q_dram[:, 4 * g:4 * g + 4, :])

            # per-b channel norms:  norm2[b, t, d] = sum_c query^2
            norm_ps = [pp_norm.tile([32, D], f32, tag=f"n{b}", name=f"norm{b}")
                       for b in range(BPC)]
            for b in range(BPC):
                for lt in range(TPB):
                    i = b * TPB + lt
                    sq_t = wk_sq.tile([128, D], f32r, tag="wsqr", name="sq_t")
                    if i % 2 == 0:
                        nc.scalar.activation(sq_t[:], q_res[:, i, :], ACT.Square)
                    else:
                        nc.vector.tensor_tensor(sq_t[:], q_res[:, i, :],
                                                q_res[:, i, :], OP.mult)
                    nc.tensor.matmul(norm_ps[b][:], selector[:], sq_t[:],
                                     start=(lt == 0), stop=(lt == TPB - 1))
            for b in range(BPC):
                nrm = small.tile([32, D], f32, tag="k2row", name="nrm")
                nc.scalar.sqrt(nrm[:], norm_ps[b][:])
                nc.vector.tensor_scalar_max(nrm[:], nrm[:], 1e-12)
                nc.vector.reciprocal(nrm[:], nrm[:])
                for k in range(4):
                    if k < 2:
                        nc.scalar.copy(inv_rep[32 * k:32 * k + 32, b, :], nrm[:])
                    else:
                        nc.vector.tensor_copy(inv_rep[32 * k:32 * k + 32, b, :], nrm[:])

        pp_a = ctx.enter_context(tc.tile_pool(name="pp_a", bufs=1, space="PSUM"))
        pp_b = ctx.enter_context(tc.tile_pool(name="pp_b", bufs=2, space="PSUM"))
        pp_seg = ctx.enter_context(tc.tile_pool(name="pp_seg", bufs=1, space="PSUM"))

        # ================= PHASE 1 (4-tile groups) =================
        colsum_ps = pp_cs.tile([1, M], f32, tag="cs")
        for g in range(TN // 4):
            for t in range(4):
                i = 4 * g + t
                b = i // TPB
                # qn overwrites the raw query rows in place
                nc.vector.scalar_tensor_tensor(q_res[:, i, :], q_res[:, i, :],
                                               1.0, inv_rep[:, b, :],
                                               OP.mult, OP.mult)
                qnT_ps = pp_a.tile([128, D], f32, tag="pa", name="qnT_ps")
                for jd in range(4):
                    nc.tensor.transpose(qnT_ps[:, jd * 128:(jd + 1) * 128],
                                        q_res[:, i, jd * 128:(jd + 1) * 128],
                                        identity[:])
                qnT_sb = wk_tr.tile([128, 4, 128], f32r, tag="wtr", name="qnT_sb")
                for jd in range(4):
                    nc.scalar.copy(qnT_sb[:, jd, :],
                                   qnT_ps[:, jd * 128:(jd + 1) * 128])
                score_ps = pp_b.tile([128, M], f32, tag="pb", name="score_ps")
                for jd in range(4):
                    nc.tensor.matmul(score_ps[:], qnT_sb[:, jd, :],
                                     keysT[:, jd, :],
                                     start=(jd == 0), stop=(jd == 3))
                nc.scalar.activation(E_res[:, i, :], score_ps[:], ACT.Exp,
                                     accum_out=rowinv_all[:, i:i + 1])
                nc.tensor.matmul(colsum_ps[:], ones_bf[:], E_res[:, i, :],
                                 start=(i == 0), stop=(i == TN - 1),
                                 skip_group_check=True)
                nc.vector.max(vmax_all[:, i, :], E_res[:, i, :])
                nc.vector.max_index(vidx_all[:, i, :], vmax_all[:, i, :],
                                    E_res[:, i, :])
                nc.vector.tensor_tensor(colmax_acc[:], colmax_acc[:],
                                        E_res[:, i, :], OP.max)
                sq2 = wk_sq.tile([128, D], f32, tag="wsq", name="sq2")
                nc.scalar.activation(sq2[:], q_res[:, i, :], ACT.Square,
                                     accum_out=qrn2_all[:, i:i + 1])
            # batched reciprocal + sm for the group
            nc.vector.reciprocal(rowinv_all[:, 4 * g:4 * g + 4],
                                 rowinv_all[:, 4 * g:4 * g + 4])
            sm_stage = st_sm.tile([128, 4, M], f32, tag="ssm", name="sm_stage")
            for t in range(4):
                i = 4 * g + t
                nc.scalar.activation(sm_stage[:, t, :], E_res[:, i, :], ACT.Copy,
                                     scale=rowinv_all[:, i:i + 1])
            nc.sync.dma_start(sm_dram[:, 4 * g:4 * g + 4, :], sm_stage[:])
        # colsum -> cc2 (column layout: m = 128j + p at [p, 2052+j])
        csrow = small.tile([1, M], f32)
        nc.scalar.copy(csrow[:], colsum_ps[:])
        nc.sync.dma_start(cc2_in[262656:263168].rearrange("(x m) -> x m", x=1),
                          csrow[:])
        # column max of E (partition reduce; collective after AR2)
        nc.gpsimd.partition_all_reduce(colmax_f[:], colmax_acc[:], 128,
                                       bass_isa.ReduceOp.max)
        cmrow = small.tile([1, M], f32)
        nc.vector.tensor_copy(cmrow[:], colmax_f[0:1, :])
        nc.sync.dma_start(cc3_in[:], cmrow[:])

        # ================= PHASE 2a-seg: segsum first (early AR2) ==========
        seg_ps = [pp_seg.tile([128, D], f32, tag=f"sg{j}", name=f"seg{j}")
                  for j in range(4)]
        for i in range(TN):
            oh1k = wk_oh.tile([128, M], bf16, tag="woh", name="oh1k")
            nc.vector.scalar_tensor_tensor(oh1k[:], iota_u[:],
                                           vidx_all[:, i, 0:1], keys2_bc[:],
                                           OP.is_equal, OP.mult,
                                           accum_out=A1_all[:, i:i + 1])
            oh2k = wk_oh.tile([128, M], bf16, tag="woh", name="oh2k")
            nc.vector.scalar_tensor_tensor(oh2k[:], iota_u[:],
                                           vidx_all[:, i, 1:2], keys2_bc[:],
                                           OP.is_equal, OP.mult,
                                           accum_out=A2_all[:, i:i + 1])
            # wqr = vmax0 * qn   (qn lives in q_res now)
            wqr = wk_wqr.tile([128, D], bf16, tag="wwqr", name="wqr")
            nc.vector.tensor_scalar_mul(wqr[:], q_res[:, i, :],
                                        vmax_all[:, i, 0:1])
            for jm in range(4):
                nc.tensor.matmul(seg_ps[jm][:], oh1k[:, jm * 128:(jm + 1) * 128],
                                 wqr[:], start=(i == 0), stop=(i == TN - 1),
                                 skip_group_check=True)

        # ---- batched losses (A1/A2 complete) ----
        lnv = small.tile([128, TN, 2], f32)
        nc.scalar.activation(lnv[:], vmax_all[:, :, 0:2], ACT.Ln)
        dp2 = small.tile([128, TN], f32)
        dn2 = small.tile([128, TN], f32)
        nc.vector.scalar_tensor_tensor(dp2[:], lnv[:, :, 0], -2.0, A1_all[:],
                                       OP.mult, OP.add)
        nc.vector.tensor_tensor(dp2[:], dp2[:], qrn2_all[:], OP.add)
        nc.vector.scalar_tensor_tensor(dn2[:], lnv[:, :, 1], -2.0, A2_all[:],
                                       OP.mult, OP.add)
        nc.vector.tensor_tensor(dn2[:], dn2[:], qrn2_all[:], OP.add)
        cscr = small.tile([128, TN], f32)
        nc.vector.tensor_scalar(cscr[:], dp2[:], 1.0, 0.0, OP.mult, OP.add,
                                accum_out=lossb[:, 1:2])
        nc.scalar.sqrt(dp2[:], dp2[:])
        nc.scalar.sqrt(dn2[:], dn2[:])
        nc.vector.scalar_tensor_tensor(cscr[:], dn2[:], -1.0, dp2[:],
                                       OP.mult, OP.add)
        sscr = small.tile([128, TN], f32)
        nc.scalar.activation(sscr[:], cscr[:], ACT.Relu, bias=1.0,
                             accum_out=lossb[:, 0:1])

        # ---- AR2: segsum + losses (overlaps the uq sweep below) ----
        seg_sb = q_res[:, 0:4, :]
        for jm in range(4):
            if jm < 2:
                nc.scalar.copy(seg_sb[:, jm, :], seg_ps[jm][:])
            else:
                nc.vector.tensor_copy(seg_sb[:, jm, :], seg_ps[jm][:])
        nc.sync.dma_start(cc2_in_v[:, 0:2048], seg_sb[:])
        nc.sync.dma_start(cc2_in_v[:, 2048:2050], lossb[:])
        nc.gpsimd.collective_compute("AllReduce", OP.add, replica_groups=RG,
                                     ins=[cc2_in[:].opt()], outs=[cc2_out[:].opt()])
        # colinv broadcast sits between the collectives on the GpSimd queue so
        # the sq sweep is not gated behind AR3
        nc.sync.dma_start(csrow[:],
                          cc2_out[262656:263168].rearrange("(x m) -> x m", x=1))
        nc.vector.reciprocal(csrow[:], csrow[:])
        nc.gpsimd.partition_broadcast(colinv_bc[:], csrow[:])
        nc.gpsimd.collective_compute("AllReduce", OP.max, replica_groups=RG,
                                     ins=[cc3_in[:].opt()], outs=[cc3_out[:].opt()])

        # ================= PHASE 2a-uq: attention read =================
        for g in range(TN // 4):
            uq_stage = st_uq.tile([128, 4, D], f32, tag="suq", name="uq_stage")
            for t in range(4):
                i = 4 * g + t
                ET_ps = pp_a.tile([128, M], bf16, tag="pa", name="ET_ps")
                for jm in range(4):
                    nc.tensor.transpose(ET_ps[:, jm * 128:(jm + 1) * 128],
                                        E_res[:, i, jm * 128:(jm + 1) * 128],
                                        identity_bf[:])
                ET_sb = wk_tr.tile([128, 4, 128], bf16, tag="wtr2", name="ET_sb")
                for jm in range(4):
                    if jm < 2:
                        nc.scalar.copy(ET_sb[:, jm, :],
                                       ET_ps[:, jm * 128:(jm + 1) * 128])
                    else:
                        nc.vector.tensor_copy(ET_sb[:, jm, :],
                                              ET_ps[:, jm * 128:(jm + 1) * 128])
                uq_ps = pp_b.tile([128, D], f32, tag="pb", name="uq_ps")
                for jm in range(4):
                    nc.tensor.matmul(uq_ps[:], ET_sb[:, jm, :], k_nbf[:, jm, :],
                                     start=(jm == 0), stop=(jm == 3))
                nc.scalar.activation(uq_stage[:, t, :], uq_ps[:], ACT.Copy,
                                     scale=rowinv_all[:, i:i + 1])
            nc.sync.dma_start(uq_dram[:, 4 * g:4 * g + 4, :], uq_stage[:])

        # ================= PHASE 2b: score_query out =================
        sq_stage = None
        for i in range(TN):
            if i % 4 == 0:
                sq_stage = st_sm.tile([128, 4, M], f32, tag="ssm", name="sq_stage")
            nc.vector.tensor_tensor(sq_stage[:, i % 4, :], E_res[:, i, :],
                                    colinv_bc[:], OP.mult)
            if i % 4 == 3:
                nc.sync.dma_start(sq_dram[:, i - 3:i + 1, :], sq_stage[:])

        # ================= TAIL: updated_memory + losses =================
        nc.sync.dma_start(colmax_col[:],
                          cc3_out[:].rearrange("x (j p) -> p (x j)", p=128))
        # rescale = 1/(keys2[m] * colmaxE[m])
        nc.vector.tensor_tensor(colmax_col[:], colmax_col[:], keys2_col[:], OP.mult)
        nc.vector.reciprocal(colmax_col[:], colmax_col[:])
        seg_glob = q_res[:, 4:8, :]
        nc.sync.dma_start(seg_glob[:], cc2_out_v[:, 0:2048])
        umn = small.tile([128, 4], f32)
        for jm in range(4):
            nc.vector.scalar_tensor_tensor(seg_glob[:, jm, :], seg_glob[:, jm, :],
                                           colmax_col[:, jm:jm + 1],
                                           k_nat[:, jm, :].bitcast(f32),
                                           OP.mult, OP.add)
            umscr = wk_sq.tile([128, D], f32, tag="wsq", name="umscr")
            nc.scalar.activation(umscr[:], seg_glob[:, jm, :], ACT.Square,
                                 accum_out=umn[:, jm:jm + 1])
        nc.scalar.sqrt(umn[:], umn[:])
        nc.vector.tensor_scalar_max(umn[:], umn[:], 1e-12)
        nc.vector.reciprocal(umn[:], umn[:])
        for jm in range(4):
            nc.vector.tensor_scalar_mul(seg_glob[:, jm, :], seg_glob[:, jm, :],
                                        umn[:, jm:jm + 1])
        nc.sync.dma_start(um_dram[:], seg_glob[:])

        # global losses
        nc.sync.dma_start(lossb[:], cc2_out_v[:, 2048:2050])
        nc.gpsimd.partition_all_reduce(lossb[:], lossb[:], 128,
                                       bass_isa.ReduceOp.add)
        lrow = small.tile([1, 2], f32)
        nc.vector.tensor_copy(lrow[:], lossb[0:1, :])
        nc.sync.dma_start(loss_out.ap()[:], lrow[:])


_CACHE = {}


def _build():
    if "nc" in _CACHE:
        return _CACHE["nc"]
    nc = bacc.Bacc("TRN2", target_bir_lowering=False, debug=False,
                   num_devices=NCORES)
    q_in = nc.dram_tensor("q", [NSH, D], f32, kind="ExternalInput")
    k_in = nc.dram_tensor("k", [M, D], f32, kind="ExternalInput")
    sm_out = nc.dram_tensor("sm", [NSH, M], f32, kind="ExternalOutput")
    sq_out = nc.dram_tensor("sq", [NSH, M], f32, kind="ExternalOutput")
    uq_out = nc.dram_tensor("uq", [NSH, D], f32, kind="ExternalOutput")
    um_out = nc.dram_tensor("um", [M, D], f32, kind="ExternalOutput")
    loss_out = nc.dram_tensor("loss", [1, 2], f32, kind="ExternalOutput")
    with tile.TileContext(nc) as tc:
        _emit(tc, q_in, k_in, sm_out, sq_out, uq_out, um_out, loss_out)
    nc.compile()
    _CACHE["nc"] = nc
    return nc


def run(query, keys, trace=False, **trace_kw):
    nc = _build()
    qr = np.ascontiguousarray(np.asarray(query, np.float32).reshape(N, D))
    kk = np.ascontiguousarray(np.asarray(keys, np.float32))
    in_maps = [{"q": qr[c * NSH:(c + 1) * NSH], "k": kk} for c in range(NCORES)]
    res = run_bass_kernel_spmd(nc, in_maps, core_ids=list(range(NCORES)),
                               trace=trace, **trace_kw)
    return res


def kernel(query, keys):
    res = run(query, keys)
    outs = res.results
    uq = np.concatenate([outs[c]["uq"] for c in range(NCORES)], axis=0)
    sm = np.concatenate([outs[c]["sm"] for c in range(NCORES)], axis=0)
    sq = np.concatenate([outs[c]["sq"] for c in range(NCORES)], axis=0)
    um = outs[0]["um"]
    loss = outs[0]["loss"].reshape(2)
    updated_query = uq.reshape(B, C, T, D)
    separateness = np.float32(loss[0] / N)
    compactness = np.float32(loss[1] / (N * D))
    return (updated_query, um, sq, sm, separateness, compactness)
